# revision 3
# baseline (speedup 1.0000x reference)
"""Trainium2 Bass kernel for nn_Block_16174846837078 (moe_routing), v2.

Data-parallel over batch: each of the 8 cores gets 4 "large"-half and 4
"small"-half samples. Single NEFF, all compute on-device.

v2 structural changes over the baseline:
  - All weight DMAs issued early on the gpsimd ring (wv/wqk/wproj during
    LN1, w1/w2 during attention tail) so no phase ever stalls on HBM.
  - x loaded in bulk per sample (no per-chunk DMA on the critical path).
  - No DRAM scratch roundtrips: x2 and xn2T live in SBUF (x2 in bf16).
  - Attention inner loop software-pipelined: QK(i) | PV(i-1) | pR(i-2)
    back-to-back on the PE so it stays at full clock; exp batched to one
    activation per (s,head); the 257th-key scores batched across samples.
  - Softmax reciprocal on the vector engine (no Ln/Exp act-table thrash).
  - MLP software-pipelined: fc1(m+1) emitted before fc2(m) so the gelu
    hides under matmuls; half-1 prefix snapshots copied out by the vector
    engine so fc1's accumulation never waits on a gelu.
"""

import numpy as np

P = 128
H = 12
HD = 64
C = 768
HID = 3072
N = 257
SL = 4              # large samples per core
SS = 4              # small samples per core
T = SL * N          # 1028 tokens per half per core
NCORES = 8
EPS = 1e-5

TCH = [(o, min(P, T - o)) for o in range(0, T, P)]          # 8x128 + 1x4
KCH = [(0, 128), (128, 128), (256, 1)]
NP = N + 1          # padded per-sample token count in transposed layouts
QTL = SL * NP       # 1032
CCH = [(0, 384), (384, 384)]
GRPS = [
    (0, [(0, 128), (128, 128), (256, 128)]),
    (384, [(0, 128), (128, 128), (256, 128)]),
    (768, [(0, 128), (128, 128), (256, 4)]),
]

_CACHE = {}


def _build():
    from contextlib import ExitStack

    import concourse.bacc as bacc
    import concourse.tile as tile
    from concourse import mybir
    from concourse.masks import make_identity
    import concourse.bass as bass

    dt = mybir.dt
    f32 = dt.float32
    f32r = dt.bfloat16       # matmul-operand dtype
    f32red = dt.float32r     # K=1 denominator-broadcast matmul operands
    AF = mybir.ActivationFunctionType
    OP = mybir.AluOpType

    nc = bacc.Bacc("TRN2", target_bir_lowering=False, debug=False)

    # ---------------- I/O ----------------
    x_d = nc.dram_tensor("x", [SL + SS, N, C], f32, kind="ExternalInput").ap()
    gw_d = nc.dram_tensor("gumbel_weights", [3], f32, kind="ExternalInput").ap()
    vecs = {}
    for nm in ["n1l_g", "n1l_b", "n1s_g", "n1s_b", "n2l_g", "n2l_b",
               "n2s_g", "n2s_b", "b_proj", "b_fc1", "b_fc2"]:
        sz = HID if nm == "b_fc1" else C
        vecs[nm] = nc.dram_tensor(nm, [sz], f32, kind="ExternalInput").ap()
    wqkv_d = nc.dram_tensor("w_qkv", [C, 3 * C], f32, kind="ExternalInput").ap()
    wproj_d = nc.dram_tensor("w_proj", [C, C], f32, kind="ExternalInput").ap()
    wfc1_d = nc.dram_tensor("w_fc1", [C, HID], f32, kind="ExternalInput").ap()
    wfc2_d = nc.dram_tensor("w_fc2", [HID, C], f32, kind="ExternalInput").ap()
    out_d = nc.dram_tensor("out", [SL + SS, N, C], f32, kind="ExternalOutput").ap()

    x_flat = x_d.rearrange("b n c -> (b n) c")
    out_flat = out_d.rearrange("b n c -> (b n) c")

    def bcast_row(vec_ap, parts=P):
        return bass.AP(tensor=vec_ap.tensor, offset=vec_ap.offset,
                       ap=[[0, parts], list(vec_ap.ap[0])])

    with tile.TileContext(nc) as tc, \
         nc.allow_low_precision(reason="bf16 operands within rel tolerance"):
        with tc.tile_pool(name="const", bufs=1) as const:
            ident = const.tile([P, P], f32, tag="ident")
            make_identity(nc, ident)
            ident_b = const.tile([P, P], f32r, tag="ident_b")
            nc.vector.tensor_copy(ident_b, ident)
            eps_t = const.tile([P, 1], f32, tag="eps")
            nc.vector.memset(eps_t, EPS)
            ones32 = const.tile([P, P], f32, tag="ones32")
            nc.vector.memset(ones32, 1.0)
            ones_row = const.tile([1, P], f32r, tag="ones_row")
            nc.vector.tensor_copy(ones_row, ones32[0:1])
            zero24 = const.tile([P, 24], f32, tag="zero24")
            nc.vector.memset(zero24, 0.0)
            ones_t = const.tile([P, 64], f32red, tag="ones")
            nc.vector.tensor_copy(ones_t, ones32[:, 0:64])
            gw_b = const.tile([P, 3], f32, tag="gw")
            nc.gpsimd.dma_start(out=gw_b, in_=bcast_row(gw_d))
            gsum = const.tile([P, 2], f32, tag="gsum")
            nc.vector.tensor_add(gsum[:, 0:1], gw_b[:, 0:1], gw_b[:, 1:2])
            nc.vector.tensor_add(gsum[:, 1:2], gsum[:, 0:1], gw_b[:, 2:3])
            g0c = gw_b[:, 0:1]
            g1c = gw_b[:, 1:2]
            g2c = gw_b[:, 2:3]
            g01c = gsum[:, 0:1]
            g012c = gsum[:, 1:2]

            lncols = {}
            for nm in ["n1l_g", "n1l_b", "n1s_g", "n1s_b",
                       "n2l_g", "n2l_b", "n2s_g", "n2s_b"]:
                t = const.tile([P, 6], f32, tag=f"col_{nm}", name=f"col_{nm}")
                nc.sync.dma_start(out=t, in_=vecs[nm].rearrange("(j p) -> p j", p=P))
                lncols[nm] = t
            b1_col = const.tile([P, 24], f32, tag="b1col")
            nc.sync.dma_start(out=b1_col,
                              in_=vecs["b_fc1"].rearrange("(j p) -> p j", p=P))

            def small_scale3(dst, src, ranges_cols):
                for (a, b), colv in ranges_cols:
                    nc.vector.tensor_scalar_mul(dst[:, a:b], src[:, a:b], colv)

            # ---------- long-lived pools / early weight DMAs ----------
            es_oall = ExitStack()
            oall_pool = es_oall.enter_context(tc.tile_pool(name="oall", bufs=1))
            es_w0 = ExitStack()
            wp0_pool = es_w0.enter_context(tc.tile_pool(name="wp0", bufs=1))
            es_aw = ExitStack()
            wqkv_pool = es_aw.enter_context(tc.tile_pool(name="wqkv", bufs=1))
            xn_pool = es_aw.enter_context(tc.tile_pool(name="xn1", bufs=1))
            vnat_pool = es_aw.enter_context(tc.tile_pool(name="vnat", bufs=1))

            wv_sb = wqkv_pool.tile([P, 6, C], f32r, tag="wv")
            nc.gpsimd.dma_start(
                out=wv_sb,
                in_=wqkv_d[:, 2 * C:3 * C].rearrange("(j p) n -> p j n", p=P))
            wqk_sb = wqkv_pool.tile([P, 6, 2 * C], f32r, tag="wqk")
            nc.gpsimd.dma_start(
                out=wqk_sb,
                in_=wqkv_d[:, 0:2 * C].rearrange("(j p) n -> p j n", p=P))
            wproj_sb = wp0_pool.tile([P, 6, C], f32r, tag="wproj")
            nc.gpsimd.dma_start(
                out=wproj_sb, in_=wproj_d.rearrange("(j p) n -> p j n", p=P))
            bp_b = wp0_pool.tile([P, C], f32, tag="bp_b")
            nc.gpsimd.dma_start(out=bp_b, in_=bcast_row(vecs["b_proj"]))
            b2_b = wp0_pool.tile([P, C], f32, tag="b2_b")
            nc.gpsimd.dma_start(out=b2_b, in_=bcast_row(vecs["b_fc2"]))
            bp_row = wp0_pool.tile([1, C], f32r, tag="bp_row")
            nc.vector.tensor_copy(bp_row, bp_b[0:1])

            xnTs = {}
            oalls = {}
            vnats = {0: {}, 1: {}}
            for half in (0, 1):
                xnTs[half] = xn_pool.tile([P, 6, QTL], f32r,
                                          tag=f"xnT{half}", name=f"xnT{half}")
                oalls[half] = oall_pool.tile([P, 6, T], f32r,
                                             tag=f"oall{half}", name=f"oall{half}")
                for s in range(SL):
                    for kc, (kof, ksz) in enumerate(KCH):
                        vt = vnat_pool.tile([ksz, H, 65], f32r,
                                            tag=f"v{half}_{s}_{kc}",
                                            name=f"v{half}_{s}_{kc}")
                        nc.vector.tensor_copy(vt[:, :, 64:65],
                                              ones32[0:ksz, 0:H].unsqueeze(2))
                        vnats[half][(s, kc)] = vt

            # ---------------- LN1 + V, both halves ----------------
            es_ln = ExitStack()
            xh_pool = es_ln.enter_context(tc.tile_pool(name="xh", bufs=1))
            ln_pool = es_ln.enter_context(tc.tile_pool(name="ln1", bufs=3))
            pst_pool = es_ln.enter_context(
                tc.tile_pool(name="ps_t1", bufs=2, space="PSUM"))
            psv_pool = es_ln.enter_context(
                tc.tile_pool(name="ps_v", bufs=2, space="PSUM"))

            xmains = {}
            xtails = {}
            for half in (0, 1):
                xm = xh_pool.tile([P, SL, 2, C], f32, tag=f"xm{half}",
                                  name=f"xm{half}")
                for s in range(SL):
                    nc.sync.dma_start(
                        out=xm[:, s],
                        in_=x_d[half * SL + s, 0:2 * P, :].rearrange(
                            "(k p) n -> p k n", p=P))
                xt = xh_pool.tile([SL, C], f32, tag=f"xt{half}",
                                  name=f"xt{half}")
                nc.sync.dma_start(out=xt, in_=x_d[half * SL:(half + 1) * SL,
                                                  2 * P, :])
                xmains[half] = xm
                xtails[half] = xt

            def ln_chunk(src, sz, out_ap, gc, bc):
                # src: [sz, C] at base partition 0 -> LN -> 6 transposed
                # 128-channel blocks written through out_ap(j) with affine.
                xg = src.rearrange("p (g d) -> p g d", g=3)
                stats = ln_pool.tile([P, 3, 6], f32, tag="ln_stats")
                for i in range(3):
                    nc.vector.bn_stats(out=stats[0:sz, i], in_=xg[:, i])
                mv = ln_pool.tile([P, 2], f32, tag="ln_mv")
                nc.vector.bn_aggr(out=mv[0:sz], in_=stats[0:sz])
                rstd = ln_pool.tile([P, 1], f32, tag="ln_rstd")
                nc.scalar.activation(rstd[0:sz], mv[0:sz, 1:2],
                                     AF.Sqrt, bias=eps_t[0:sz], scale=1.0)
                nc.vector.reciprocal(rstd[0:sz], rstd[0:sz])
                pre = ln_pool.tile([P, C], f32r, tag="ln_pre")
                nc.vector.tensor_scalar(pre[0:sz], src,
                                        scalar1=mv[0:sz, 0:1],
                                        scalar2=rstd[0:sz],
                                        op0=OP.subtract, op1=OP.mult)
                for j in range(6):
                    pst = pst_pool.tile([P, P], f32r, tag="pst")
                    nc.tensor.transpose(pst[:, 0:sz],
                                        pre[0:sz, j * P:(j + 1) * P],
                                        ident_b[0:sz, 0:sz])
                    nc.scalar.activation(out_ap(j), pst[:, 0:sz],
                                         AF.Identity, bias=bc[:, j:j + 1],
                                         scale=gc[:, j:j + 1])

            def ln1_sample(half, s):
                gc = lncols["n1l_g" if half == 0 else "n1s_g"]
                bc = lncols["n1l_b" if half == 0 else "n1s_b"]
                xnT = xnTs[half]
                for kc in range(2):
                    of_p = s * NP + kc * P
                    ln_chunk(xmains[half][0:P, s, kc], P,
                             lambda j, o=of_p: xnT[:, j, o:o + P], gc, bc)

            def ln1_tail(half):
                # the 257th token of all 4 samples as one 4-row chunk
                gc = lncols["n1l_g" if half == 0 else "n1s_g"]
                bc = lncols["n1l_b" if half == 0 else "n1s_b"]
                xnTr = xnTs[half].rearrange("p j (s n) -> p j s n", n=NP)
                ln_chunk(xtails[half][0:SL], SL,
                         lambda j: xnTr[:, j, :, 256], gc, bc)

            def v_sample(half, s):
                xnT = xnTs[half]
                for kc, (kof, ksz) in enumerate(KCH):
                    tof = s * NP + kof
                    for ch in range(2):
                        psv = psv_pool.tile([P, 384], f32, tag="psv")
                        for kk in range(6):
                            nc.tensor.matmul(
                                psv[0:ksz],
                                lhsT=xnT[:, kk, tof:tof + ksz],
                                rhs=wv_sb[:, kk, ch * 384:(ch + 1) * 384],
                                start=(kk == 0), stop=(kk == 5))
                        nc.vector.tensor_copy(
                            vnats[half][(s, kc)][:, ch * 6:(ch + 1) * 6, 0:64],
                            psv[0:ksz].rearrange("p (h d) -> p h d", h=6))

            for half in (0, 1):
                for s in range(SL):
                    ln1_sample(half, s)
                ln1_tail(half)
                for s in range(SL):
                    v_sample(half, s)
                pads = xnTs[half].rearrange(
                    "p j (s n) -> p j s n", n=NP)[:, :, :, N:N + 1]
                nc.vector.tensor_copy(
                    pads,
                    zero24[:, 0:6 * SL].rearrange(
                        "p (j s) -> p j s", j=6).unsqueeze(3))

            es_ln.close()

            # ---------------- attention pairs, both halves ----------------
            es_at = ExitStack()
            qk_pool = es_at.enter_context(tc.tile_pool(name="qk", bufs=2))
            e_pool = es_at.enter_context(tc.tile_pool(name="epool", bufs=3))
            rec_pool = es_at.enter_context(tc.tile_pool(name="rec", bufs=3))
            sc_pool = es_at.enter_context(
                tc.tile_pool(name="ps_sc", bufs=2, space="PSUM"))
            acc_pool = es_at.enter_context(
                tc.tile_pool(name="ps_acc", bufs=2, space="PSUM"))
            aux_pool = es_at.enter_context(
                tc.tile_pool(name="ps_aux", bufs=2, space="PSUM"))

            def pairs(half):
                xnT = xnTs[half]
                oall = oalls[half]
                vnat = vnats[half]
                for pair in range(6):
                    qT = qk_pool.tile([P, QTL], f32r, tag="qT")
                    kT = qk_pool.tile([P, QTL], f32r, tag="kT")
                    for dst, cbase in ((qT, pair * P), (kT, C + pair * P)):
                        for s in range(SL):
                            sct = sc_pool.tile([P, 1024], f32, tag="sc")
                            pq = sct[:, 0:NP]
                            for kk in range(6):
                                nc.tensor.matmul(
                                    pq,
                                    lhsT=wqk_sb[:, kk, cbase:cbase + P],
                                    rhs=xnT[:, kk, s * NP:(s + 1) * NP],
                                    start=(kk == 0), stop=(kk == 5))
                            nc.vector.tensor_copy(dst[:, s * NP:(s + 1) * NP], pq)
                    # 257th-key scores: one [1, NP] row per (head, sample)
                    kTr = kT.rearrange("p (s n) -> p s n", n=NP)
                    et1s = {}
                    for hh in range(2):
                        rlo, rhi = hh * 64, hh * 64 + 64
                        for s in range(SL):
                            sk = sc_pool.tile([P, 1024], f32, tag="sc")
                            nc.tensor.matmul(
                                sk[0:1, 0:NP],
                                lhsT=kTr[rlo:rhi, s, 256:257],
                                rhs=qT[rlo:rhi, s * NP:(s + 1) * NP],
                                start=True, stop=True)
                            et1 = e_pool.tile([1, NP], f32r, tag="et1",
                                              bufs=8)
                            nc.scalar.activation(et1, sk[0:1, 0:NP],
                                                 AF.Exp, scale=HD ** -0.5)
                            et1s[(hh, s)] = et1

                    def emit_pv(ent):
                        s, hh, et = ent
                        h = 2 * pair + hh
                        po = acc_pool.tile([65, NP], f32, tag="po")
                        nc.tensor.matmul(po, lhsT=vnat[(s, 0)][:, h, :],
                                         rhs=et[:, 0, :], start=True, stop=False)
                        nc.tensor.matmul(po, lhsT=vnat[(s, 1)][:, h, :],
                                         rhs=et[:, 1, :], start=False, stop=False)
                        nc.tensor.matmul(po, lhsT=vnat[(s, 2)][:, h, :],
                                         rhs=et1s[(hh, s)],
                                         start=False, stop=True)
                        rec = rec_pool.tile([65, NP], f32red, tag="rec")
                        nc.vector.reciprocal(rec[64:65], po[64:65])
                        osl = oall[hh * 64:hh * 64 + 64, pair,
                                   s * N:(s + 1) * N]
                        nc.vector.tensor_copy(osl, po[0:64, 0:N])
                        return (osl, rec)

                    def emit_norm(ent):
                        osl, rec = ent
                        pR = aux_pool.tile([64, NP], f32, tag="pR")
                        nc.tensor.matmul(pR, lhsT=ones_t[64:65, 0:64],
                                         rhs=rec[64:65], start=True, stop=True)
                        nc.vector.tensor_mul(osl, osl, pR[:, 0:N])

                    pv_q = None
                    norm_q = None
                    for s in range(SL):
                        for hh in range(2):
                            rlo, rhi = hh * 64, hh * 64 + 64
                            sct = sc_pool.tile([P, 1024], f32, tag="sc")
                            nc.tensor.matmul(
                                sct[:, 0:NP],
                                lhsT=kT[rlo:rhi, s * NP:s * NP + 128],
                                rhs=qT[rlo:rhi, s * NP:(s + 1) * NP],
                                start=True, stop=True)
                            nc.tensor.matmul(
                                sct[:, 512:512 + NP],
                                lhsT=kT[rlo:rhi, s * NP + 128:s * NP + 256],
                                rhs=qT[rlo:rhi, s * NP:(s + 1) * NP],
                                start=True, stop=True)
                            et = e_pool.tile([P, 2, NP], f32r, tag="et")
                            nc.scalar.activation(
                                et,
                                sct.rearrange("p (a b) -> p a b", a=2)[:, :, 0:NP],
                                AF.Exp, scale=HD ** -0.5)
                            if pv_q is not None:
                                ent = emit_pv(pv_q)
                                if norm_q is not None:
                                    emit_norm(norm_q)
                                norm_q = ent
                            pv_q = (s, hh, et)
                    ent = emit_pv(pv_q)
                    if norm_q is not None:
                        emit_norm(norm_q)
                    emit_norm(ent)

            pairs(0)
            pairs(1)
            es_at.close()
            es_aw.close()

            # ---------------- proj + LN2, both halves ----------------
            es_late = ExitStack()
            wm_pool = es_late.enter_context(tc.tile_pool(name="wmlp", bufs=1))
            x2_pool = es_late.enter_context(tc.tile_pool(name="x2sb", bufs=1))
            xn2_pool = es_late.enter_context(tc.tile_pool(name="xn2sb", bufs=1))

            w1_sb = wm_pool.tile([P, 6, HID], f32r, tag="w1")
            nc.gpsimd.dma_start(out=w1_sb,
                                in_=wfc1_d.rearrange("(j p) n -> p j n", p=P))
            w2_sb = wm_pool.tile([P, 24, C], f32r, tag="w2")
            nc.gpsimd.dma_start(out=w2_sb,
                                in_=wfc2_d.rearrange("(j p) n -> p j n", p=P))
            b2t_b = wm_pool.tile([P, C], f32, tag="b2t_b")
            small_scale3(b2t_b, b2_b,
                         [((0, 256), g012c), ((256, 384), g01c),
                          ((384, 768), g0c)])

            es_proj = ExitStack()
            wp1_pool = es_proj.enter_context(tc.tile_pool(name="wp1", bufs=1))
            pr_pool = es_proj.enter_context(tc.tile_pool(name="prtmp", bufs=3))
            psp_pool = es_proj.enter_context(
                tc.tile_pool(name="ps_p", bufs=4, space="PSUM"))
            ln2_pool = es_proj.enter_context(tc.tile_pool(name="ln2", bufs=3))
            pst2_pool = es_proj.enter_context(
                tc.tile_pool(name="ps_t2", bufs=2, space="PSUM"))

            wt_sb = wp1_pool.tile([P, 6, C], f32r, tag="wtilde")
            for j in range(6):
                if j < 2:
                    small_scale3(wt_sb[:, j], wproj_sb[:, j],
                                 [((0, 256), g012c), ((256, 384), g01c),
                                  ((384, 768), g0c)])
                elif j == 2:
                    small_scale3(wt_sb[:, j], wproj_sb[:, j],
                                 [((0, 384), g01c), ((384, 768), g0c)])
                else:
                    small_scale3(wt_sb[:, j], wproj_sb[:, j],
                                 [((0, 768), g0c)])
            bt_b = wp1_pool.tile([P, C], f32, tag="btilde")
            small_scale3(bt_b, bp_b,
                         [((0, 256), g012c), ((256, 384), g01c),
                          ((384, 768), g0c)])
            bt_row = wp1_pool.tile([1, C], f32r, tag="bt_row")
            nc.vector.tensor_copy(bt_row, bt_b[0:1])

            x2alls = {}
            for half in (0, 1):
                x2alls[half] = x2_pool.tile([P, len(TCH), C], f32r,
                                            tag=f"x2all{half}",
                                            name=f"x2all{half}")
            xn2Ts = {}

            def proj_mm(half):
                oall = oalls[half]
                wp_eff = wproj_sb if half == 0 else wt_sb
                bp_eff = bp_row if half == 0 else bt_row
                x2all = x2alls[half]
                for i, (of, sz) in enumerate(TCH):
                    x_t = pr_pool.tile([P, C], f32, tag="resx")
                    nc.sync.dma_start(
                        out=x_t[0:sz, 0:384],
                        in_=x_flat[half * T + of:half * T + of + sz, 0:384])
                    nc.scalar.dma_start(
                        out=x_t[0:sz, 384:768],
                        in_=x_flat[half * T + of:half * T + of + sz, 384:768])
                    for ch, (ca, cw) in enumerate(CCH):
                        pp = psp_pool.tile([P, 384], f32, tag="psp")
                        for dk in range(6):
                            nc.tensor.matmul(
                                pp[0:sz],
                                lhsT=oall[:, dk, of:of + sz],
                                rhs=wp_eff[:, dk, ca:ca + cw],
                                start=(dk == 0), stop=False)
                        nc.tensor.matmul(
                            pp[0:sz], lhsT=ones_row[:, 0:sz],
                            rhs=bp_eff[:, ca:ca + cw],
                            start=False, stop=True)
                        nc.vector.tensor_add(x2all[0:sz, i, ca:ca + cw],
                                             pp[0:sz], x_t[0:sz, ca:ca + cw])

            def ln2_phase(half):
                gc = lncols["n2l_g" if half == 0 else "n2s_g"]
                bc = lncols["n2l_b" if half == 0 else "n2s_b"]
                xn2T = xn2_pool.tile([P, 6, T], f32r, tag=f"xn2T{half}",
                                     name=f"xn2T{half}")
                xn2Ts[half] = xn2T
                for i, (of, sz) in enumerate(TCH):
                    x2_t = x2alls[half][0:sz, i]
                    xg2 = x2_t.rearrange("p (g d) -> p g d", g=3)
                    stats = ln2_pool.tile([P, 3, 6], f32, tag="ln2_stats")
                    for gi in range(3):
                        nc.vector.bn_stats(out=stats[0:sz, gi], in_=xg2[:, gi])
                    mv = ln2_pool.tile([P, 2], f32, tag="ln2_mv")
                    nc.vector.bn_aggr(out=mv[0:sz], in_=stats[0:sz])
                    rstd = ln2_pool.tile([P, 1], f32, tag="ln2_rstd")
                    nc.scalar.activation(rstd[0:sz], mv[0:sz, 1:2], AF.Sqrt,
                                         bias=eps_t[0:sz], scale=1.0)
                    nc.vector.reciprocal(rstd[0:sz], rstd[0:sz])
                    pre = ln2_pool.tile([P, C], f32r, tag="ln2_pre")
                    nc.vector.tensor_scalar(pre[0:sz], x2_t,
                                            scalar1=mv[0:sz, 0:1],
                                            scalar2=rstd[0:sz],
                                            op0=OP.subtract, op1=OP.mult)
                    for j in range(6):
                        pst = pst2_pool.tile([P, P], f32r, tag="pst2")
                        nc.tensor.transpose(pst[:, 0:sz],
                                            pre[0:sz, j * P:(j + 1) * P],
                                            ident_b[0:sz, 0:sz])
                        nc.scalar.activation(xn2T[:, j, of:of + sz],
                                             pst[:, 0:sz], AF.Identity,
                                             bias=bc[:, j:j + 1],
                                             scale=gc[:, j:j + 1])

            proj_mm(0)
            ln2_phase(0)
            proj_mm(1)
            ln2_phase(1)
            es_proj.close()

            # ---------------- MLP, both halves ----------------
            es_mlp = ExitStack()
            hs_pool = es_mlp.enter_context(tc.tile_pool(name="hsm", bufs=2))
            mo_pool = es_mlp.enter_context(tc.tile_pool(name="mout", bufs=3))
            psf_pool = es_mlp.enter_context(
                tc.tile_pool(name="ps_f", bufs=2, space="PSUM"))
            psout_pool = es_mlp.enter_context(
                tc.tile_pool(name="ps_out", bufs=1, space="PSUM"))

            for half in (0, 1):
                b2_eff = b2_b if half == 0 else b2t_b
                xn2T = xn2Ts[half]
                for (gof, chunks) in GRPS:
                    gsz = sum(s for _, s in chunks)
                    xg = xn2T[:, :, gof:gof + gsz]
                    pso = [[psout_pool.tile([P, 384], f32, tag=f"pso_{i}_{ch}",
                                            name=f"pso_{i}_{ch}")
                            for ch in range(2)] for i in range(len(chunks))]

                    def emit_fc2(hstuff, m):
                        if half == 0:
                            hrow = hstuff
                            for i, (tco, tcs) in enumerate(chunks):
                                for ch, (ca, cw) in enumerate(CCH):
                                    nc.tensor.matmul(
                                        pso[i][ch][0:tcs],
                                        lhsT=hrow[:, tco:tco + tcs],
                                        rhs=w2_sb[:, m, ca:ca + cw],
                                        start=(m == 0), stop=(m == 23))
                        else:
                            HAt, HBt, h0t = hstuff
                            for i, (tco, tcs) in enumerate(chunks):
                                tsl = slice(tco, tco + tcs)
                                # HBt shares a PSUM bank with HAt's region;
                                # only the first (HAt) matmul may start it.
                                for src, ch, ca, cw, st in (
                                        (HAt, 0, 0, 256, m == 0),
                                        (HBt, 0, 256, 128, False),
                                        (h0t, 1, 384, 384, m == 0)):
                                    nc.tensor.matmul(
                                        pso[i][ch][0:tcs, ca - ch * 384:
                                                   ca - ch * 384 + cw],
                                        lhsT=src[:, tsl],
                                        rhs=w2_sb[:, m, ca:ca + cw],
                                        start=st, stop=(m == 23),
                                        skip_group_check=True)

                    hprev = None
                    for m in range(24):
                        msl = slice(m * P, (m + 1) * P)
                        pf = psf_pool.tile([P, 384], f32, tag="psf")
                        if half == 0:
                            for kk in range(6):
                                nc.tensor.matmul(
                                    pf[:, 0:gsz],
                                    lhsT=w1_sb[:, kk, msl],
                                    rhs=xg[:, kk],
                                    start=(kk == 0), stop=(kk == 5))
                            hrow = hs_pool.tile([P, 384], f32r, tag="hrow")
                            nc.scalar.activation(hrow[:, 0:gsz], pf[:, 0:gsz],
                                                 AF.Gelu,
                                                 bias=b1_col[:, m:m + 1],
                                                 scale=1.0)
                            cur = hrow
                        else:
                            c2 = hs_pool.tile([P, 384], f32, tag="c2")
                            c1 = hs_pool.tile([P, 384], f32, tag="c1")
                            for kk in range(6):
                                nc.tensor.matmul(
                                    pf[:, 0:gsz],
                                    lhsT=w1_sb[:, kk, msl],
                                    rhs=xg[:, kk],
                                    start=(kk == 0),
                                    stop=(kk in (1, 2, 5)),
                                    skip_group_check=(kk >= 2))
                                if kk == 1:
                                    nc.vector.tensor_copy(c2[:, 0:gsz],
                                                          pf[:, 0:gsz])
                                elif kk == 2:
                                    nc.vector.tensor_copy(c1[:, 0:gsz],
                                                          pf[:, 0:gsz])
                            h2t = hs_pool.tile([P, 384], f32r, tag="h2t")
                            nc.scalar.activation(h2t[:, 0:gsz], c2[:, 0:gsz],
                                                 AF.Gelu,
                                                 bias=b1_col[:, m:m + 1],
                                                 scale=1.0)
                            h1t = hs_pool.tile([P, 384], f32r, tag="h1t")
                            nc.scalar.activation(h1t[:, 0:gsz], c1[:, 0:gsz],
                                                 AF.Gelu,
                                                 bias=b1_col[:, m:m + 1],
                                                 scale=1.0)
                            h0t = hs_pool.tile([P, 384], f32r, tag="h0t")
                            nc.scalar.activation(h0t[:, 0:gsz], pf[:, 0:gsz],
                                                 AF.Gelu,
                                                 bias=b1_col[:, m:m + 1],
                                                 scale=1.0)
                            nc.vector.tensor_scalar_mul(h0t[:, 0:gsz],
                                                        h0t[:, 0:gsz], g0c)
                            HBt = hs_pool.tile([P, 384], f32r, tag="HBt")
                            nc.vector.scalar_tensor_tensor(
                                HBt[:, 0:gsz], in0=h1t[:, 0:gsz], scalar=g1c,
                                in1=h0t[:, 0:gsz], op0=OP.mult, op1=OP.add)
                            HAt = hs_pool.tile([P, 384], f32r, tag="HAt")
                            nc.vector.scalar_tensor_tensor(
                                HAt[:, 0:gsz], in0=h2t[:, 0:gsz], scalar=g2c,
                                in1=HBt[:, 0:gsz], op0=OP.mult, op1=OP.add)
                            cur = (HAt, HBt, h0t)
                        if hprev is not None:
                            emit_fc2(hprev[0], hprev[1])
                        hprev = (cur, m)
                    emit_fc2(hprev[0], 23)

                    for i, (tco, tcs) in enumerate(chunks):
                        of = gof + tco
                        ig = of // P
                        ev = mo_pool.tile([P, C], f32, tag="mo_ev")
                        for ch, (ca, cw) in enumerate(CCH):
                            nc.vector.tensor_add(ev[0:tcs, ca:ca + cw],
                                                 pso[i][ch][0:tcs],
                                                 b2_eff[0:tcs, ca:ca + cw])
                        ot = mo_pool.tile([P, C], f32, tag="mo_out")
                        nc.vector.tensor_add(ot[0:tcs], ev[0:tcs],
                                             x2alls[half][0:tcs, ig])
                        eng = (nc.sync, nc.scalar, nc.gpsimd)[i % 3]
                        eng.dma_start(
                            out=out_flat[half * T + of:half * T + of + tcs],
                            in_=ot[0:tcs])

            es_mlp.close()
            es_late.close()
            es_w0.close()
            es_oall.close()

    nc.compile()
    return nc


def _get_nc():
    if "nc" not in _CACHE:
        _CACHE["nc"] = _build()
    return _CACHE["nc"]


def kernel(**inputs):
    from concourse import bass_utils

    nc = _get_nc()
    arrs = {k: np.ascontiguousarray(np.asarray(v, dtype=np.float32))
            for k, v in inputs.items()}
    x = arrs.pop("x")
    B = x.shape[0]
    B2 = B // 2
    per = B2 // NCORES
    in_maps = []
    for c in range(NCORES):
        shard = np.concatenate([x[c * per:(c + 1) * per],
                                x[B2 + c * per:B2 + (c + 1) * per]], axis=0)
        m = {"x": np.ascontiguousarray(shard)}
        m.update(arrs)
        in_maps.append(m)
    res = bass_utils.run_bass_kernel_spmd(nc, in_maps,
                                          core_ids=list(range(NCORES)))
    out = np.empty_like(x.reshape(B, N, C))
    for c in range(NCORES):
        o = res.results[c]["out"]
        out[c * per:(c + 1) * per] = o[:per]
        out[B2 + c * per:B2 + (c + 1) * per] = o[per:]
    return out


# revision 4
# speedup vs baseline: 1.0815x; 1.0815x over previous
"""Trainium2 Bass kernel for nn_Block_16174846837078 (moe_routing), v2.

Data-parallel over batch: each of the 8 cores gets 4 "large"-half and 4
"small"-half samples. Single NEFF, all compute on-device.

v2 structural changes over the baseline:
  - All weight DMAs issued early on the gpsimd ring (wv/wqk/wproj during
    LN1, w1/w2 during attention tail) so no phase ever stalls on HBM.
  - x loaded in bulk per sample (no per-chunk DMA on the critical path).
  - No DRAM scratch roundtrips: x2 and xn2T live in SBUF (x2 in bf16).
  - Attention inner loop software-pipelined: QK(i) | PV(i-1) | pR(i-2)
    back-to-back on the PE so it stays at full clock; exp batched to one
    activation per (s,head); the 257th-key scores batched across samples.
  - Softmax reciprocal on the vector engine (no Ln/Exp act-table thrash).
  - MLP software-pipelined: fc1(m+1) emitted before fc2(m) so the gelu
    hides under matmuls; half-1 prefix snapshots copied out by the vector
    engine so fc1's accumulation never waits on a gelu.
"""

import numpy as np

P = 128
H = 12
HD = 64
C = 768
HID = 3072
N = 257
SL = 4              # large samples per core
SS = 4              # small samples per core
T = SL * N          # 1028 tokens per half per core
NCORES = 8
EPS = 1e-5

TCH = [(o, min(P, T - o)) for o in range(0, T, P)]          # 8x128 + 1x4
KCH = [(0, 128), (128, 128), (256, 1)]
NP = N + 1          # padded per-sample token count in transposed layouts
QTL = SL * NP       # 1032
CCH = [(0, 384), (384, 384)]
GRPS = [
    (0, [(0, 128), (128, 128), (256, 128)]),
    (384, [(0, 128), (128, 128), (256, 128)]),
    (768, [(0, 128), (128, 128), (256, 4)]),
]

_CACHE = {}


def _build():
    from contextlib import ExitStack

    import concourse.bacc as bacc
    import concourse.tile as tile
    from concourse import mybir
    from concourse.masks import make_identity
    import concourse.bass as bass

    dt = mybir.dt
    f32 = dt.float32
    f32r = dt.bfloat16       # matmul-operand dtype
    f32red = dt.float32r     # K=1 denominator-broadcast matmul operands
    AF = mybir.ActivationFunctionType
    OP = mybir.AluOpType

    nc = bacc.Bacc("TRN2", target_bir_lowering=False, debug=False)

    # ---------------- I/O ----------------
    x_d = nc.dram_tensor("x", [SL + SS, N, C], f32, kind="ExternalInput").ap()
    gw_d = nc.dram_tensor("gumbel_weights", [3], f32, kind="ExternalInput").ap()
    vecs = {}
    for nm in ["n1l_g", "n1l_b", "n1s_g", "n1s_b", "n2l_g", "n2l_b",
               "n2s_g", "n2s_b", "b_proj", "b_fc1", "b_fc2"]:
        sz = HID if nm == "b_fc1" else C
        vecs[nm] = nc.dram_tensor(nm, [sz], f32, kind="ExternalInput").ap()
    wqkv_d = nc.dram_tensor("w_qkv", [C, 3 * C], f32, kind="ExternalInput").ap()
    wproj_d = nc.dram_tensor("w_proj", [C, C], f32, kind="ExternalInput").ap()
    wfc1_d = nc.dram_tensor("w_fc1", [C, HID], f32, kind="ExternalInput").ap()
    wfc2_d = nc.dram_tensor("w_fc2", [HID, C], f32, kind="ExternalInput").ap()
    out_d = nc.dram_tensor("out", [SL + SS, N, C], f32, kind="ExternalOutput").ap()

    x_flat = x_d.rearrange("b n c -> (b n) c")
    out_flat = out_d.rearrange("b n c -> (b n) c")

    def bcast_row(vec_ap, parts=P):
        return bass.AP(tensor=vec_ap.tensor, offset=vec_ap.offset,
                       ap=[[0, parts], list(vec_ap.ap[0])])

    with tile.TileContext(nc) as tc, \
         nc.allow_low_precision(reason="bf16 operands within rel tolerance"):
        with tc.tile_pool(name="const", bufs=1) as const:
            ident = const.tile([P, P], f32, tag="ident")
            make_identity(nc, ident)
            ident_b = const.tile([P, P], f32r, tag="ident_b")
            nc.vector.tensor_copy(ident_b, ident)
            eps_t = const.tile([P, 1], f32, tag="eps")
            nc.vector.memset(eps_t, EPS)
            ones32 = const.tile([P, P], f32, tag="ones32")
            nc.vector.memset(ones32, 1.0)
            ones_row = const.tile([1, P], f32r, tag="ones_row")
            nc.vector.tensor_copy(ones_row, ones32[0:1])
            zero24 = const.tile([P, 24], f32, tag="zero24")
            nc.vector.memset(zero24, 0.0)
            ones_t = const.tile([P, 64], f32red, tag="ones")
            nc.vector.tensor_copy(ones_t, ones32[:, 0:64])
            gw_b = const.tile([P, 3], f32, tag="gw")
            nc.gpsimd.dma_start(out=gw_b, in_=bcast_row(gw_d))
            gsum = const.tile([P, 2], f32, tag="gsum")
            nc.vector.tensor_add(gsum[:, 0:1], gw_b[:, 0:1], gw_b[:, 1:2])
            nc.vector.tensor_add(gsum[:, 1:2], gsum[:, 0:1], gw_b[:, 2:3])
            g0c = gw_b[:, 0:1]
            g1c = gw_b[:, 1:2]
            g2c = gw_b[:, 2:3]
            g01c = gsum[:, 0:1]
            g012c = gsum[:, 1:2]

            lncols = {}
            for nm in ["n1l_g", "n1l_b", "n1s_g", "n1s_b",
                       "n2l_g", "n2l_b", "n2s_g", "n2s_b"]:
                t = const.tile([P, 6], f32, tag=f"col_{nm}", name=f"col_{nm}")
                nc.sync.dma_start(out=t, in_=vecs[nm].rearrange("(j p) -> p j", p=P))
                lncols[nm] = t
            b1_col = const.tile([P, 24], f32, tag="b1col")
            nc.sync.dma_start(out=b1_col,
                              in_=vecs["b_fc1"].rearrange("(j p) -> p j", p=P))

            def small_scale3(dst, src, ranges_cols):
                for (a, b), colv in ranges_cols:
                    nc.vector.tensor_scalar_mul(dst[:, a:b], src[:, a:b], colv)

            # ---------- long-lived pools / early weight DMAs ----------
            es_oall = ExitStack()
            oall_pool = es_oall.enter_context(tc.tile_pool(name="oall", bufs=1))
            es_w0 = ExitStack()
            wp0_pool = es_w0.enter_context(tc.tile_pool(name="wp0", bufs=1))
            es_aw = ExitStack()
            wqkv_pool = es_aw.enter_context(tc.tile_pool(name="wqkv", bufs=1))
            xn_pool = es_aw.enter_context(tc.tile_pool(name="xn1", bufs=1))
            vnat_pool = es_aw.enter_context(tc.tile_pool(name="vnat", bufs=1))

            wv_sb = wqkv_pool.tile([P, 6, C], f32r, tag="wv")
            wqk_sb = wqkv_pool.tile([P, 6, 2 * C], f32r, tag="wqk")
            wproj_sb = wp0_pool.tile([P, 6, C], f32r, tag="wproj")
            bp_b = wp0_pool.tile([P, C], f32, tag="bp_b")
            b2_b = wp0_pool.tile([P, C], f32, tag="b2_b")
            bp_row = wp0_pool.tile([1, C], f32r, tag="bp_row")

            xnTs = {}
            oalls = {}
            vnats = {0: {}, 1: {}}
            for half in (0, 1):
                xnTs[half] = xn_pool.tile([P, 6, QTL], f32r,
                                          tag=f"xnT{half}", name=f"xnT{half}")
                oalls[half] = oall_pool.tile([P, 6, T], f32r,
                                             tag=f"oall{half}", name=f"oall{half}")
                for s in range(SL):
                    for kc, (kof, ksz) in enumerate(KCH):
                        vt = vnat_pool.tile([ksz, H, 65], f32r,
                                            tag=f"v{half}_{s}_{kc}",
                                            name=f"v{half}_{s}_{kc}")
                        nc.vector.tensor_copy(vt[:, :, 64:65],
                                              ones32[0:ksz, 0:H].unsqueeze(2))
                        vnats[half][(s, kc)] = vt

            # ---------------- LN1 + V, both halves ----------------
            es_ln = ExitStack()
            xh_pool = es_ln.enter_context(tc.tile_pool(name="xh", bufs=1))
            ln_pool = es_ln.enter_context(tc.tile_pool(name="ln1", bufs=3))
            pst_pool = es_ln.enter_context(
                tc.tile_pool(name="ps_t1", bufs=2, space="PSUM"))
            psv_pool = es_ln.enter_context(
                tc.tile_pool(name="ps_v", bufs=2, space="PSUM"))

            # x loads ride the gpsimd software-DGE ring (it spreads the
            # descriptors across all hardware DMA engines); issue order on
            # that ring doubles as the prefetch schedule.
            xmains = {}
            xtails = {}
            for half in (0, 1):
                xmains[half] = xh_pool.tile([P, SL, 2, C], f32,
                                            tag=f"xm{half}", name=f"xm{half}")
                xtails[half] = xh_pool.tile([SL, C], f32, tag=f"xt{half}",
                                            name=f"xt{half}")

            def issue_x(half):
                for s in range(SL):
                    nc.gpsimd.dma_start(
                        out=xmains[half][:, s],
                        in_=x_d[half * SL + s, 0:2 * P, :].rearrange(
                            "(k p) n -> p k n", p=P))
                nc.gpsimd.dma_start(
                    out=xtails[half],
                    in_=x_d[half * SL:(half + 1) * SL, 2 * P, :])

            issue_x(0)
            nc.gpsimd.dma_start(
                out=wv_sb,
                in_=wqkv_d[:, 2 * C:3 * C].rearrange("(j p) n -> p j n", p=P))
            issue_x(1)
            nc.gpsimd.dma_start(
                out=wqk_sb,
                in_=wqkv_d[:, 0:2 * C].rearrange("(j p) n -> p j n", p=P))
            nc.gpsimd.dma_start(
                out=wproj_sb, in_=wproj_d.rearrange("(j p) n -> p j n", p=P))
            nc.gpsimd.dma_start(out=bp_b, in_=bcast_row(vecs["b_proj"]))
            nc.gpsimd.dma_start(out=b2_b, in_=bcast_row(vecs["b_fc2"]))
            nc.vector.tensor_copy(bp_row, bp_b[0:1])

            def ln_chunk(src, sz, out_ap, gc, bc):
                # src: [sz, C] at base partition 0 -> LN -> 6 transposed
                # 128-channel blocks written through out_ap(j) with affine.
                xg = src.rearrange("p (g d) -> p g d", g=3)
                stats = ln_pool.tile([P, 3, 6], f32, tag="ln_stats")
                for i in range(3):
                    nc.vector.bn_stats(out=stats[0:sz, i], in_=xg[:, i])
                mv = ln_pool.tile([P, 2], f32, tag="ln_mv")
                nc.vector.bn_aggr(out=mv[0:sz], in_=stats[0:sz])
                rstd = ln_pool.tile([P, 1], f32, tag="ln_rstd")
                nc.scalar.activation(rstd[0:sz], mv[0:sz, 1:2],
                                     AF.Sqrt, bias=eps_t[0:sz], scale=1.0)
                nc.vector.reciprocal(rstd[0:sz], rstd[0:sz])
                pre = ln_pool.tile([P, C], f32r, tag="ln_pre")
                nc.vector.tensor_scalar(pre[0:sz], src,
                                        scalar1=mv[0:sz, 0:1],
                                        scalar2=rstd[0:sz],
                                        op0=OP.subtract, op1=OP.mult)
                for j in range(6):
                    pst = pst_pool.tile([P, P], f32r, tag="pst")
                    nc.tensor.transpose(pst[:, 0:sz],
                                        pre[0:sz, j * P:(j + 1) * P],
                                        ident_b[0:sz, 0:sz])
                    nc.scalar.activation(out_ap(j), pst[:, 0:sz],
                                         AF.Identity, bias=bc[:, j:j + 1],
                                         scale=gc[:, j:j + 1])

            def ln1_sample(half, s):
                gc = lncols["n1l_g" if half == 0 else "n1s_g"]
                bc = lncols["n1l_b" if half == 0 else "n1s_b"]
                xnT = xnTs[half]
                for kc in range(2):
                    of_p = s * NP + kc * P
                    ln_chunk(xmains[half][0:P, s, kc], P,
                             lambda j, o=of_p: xnT[:, j, o:o + P], gc, bc)

            def ln1_tail(half):
                # the 257th token of all 4 samples as one 4-row chunk
                gc = lncols["n1l_g" if half == 0 else "n1s_g"]
                bc = lncols["n1l_b" if half == 0 else "n1s_b"]
                xnTr = xnTs[half].rearrange("p j (s n) -> p j s n", n=NP)
                ln_chunk(xtails[half][0:SL], SL,
                         lambda j: xnTr[:, j, :, 256], gc, bc)

            def v_sample(half, s):
                xnT = xnTs[half]
                for kc, (kof, ksz) in enumerate(KCH):
                    tof = s * NP + kof
                    for ch in range(2):
                        psv = psv_pool.tile([P, 384], f32, tag="psv")
                        for kk in range(6):
                            nc.tensor.matmul(
                                psv[0:ksz],
                                lhsT=xnT[:, kk, tof:tof + ksz],
                                rhs=wv_sb[:, kk, ch * 384:(ch + 1) * 384],
                                start=(kk == 0), stop=(kk == 5))
                        nc.vector.tensor_copy(
                            vnats[half][(s, kc)][:, ch * 6:(ch + 1) * 6, 0:64],
                            psv[0:ksz].rearrange("p (h d) -> p h d", h=6))

            for half in (0, 1):
                for s in range(SL):
                    ln1_sample(half, s)
                ln1_tail(half)
                for s in range(SL):
                    v_sample(half, s)
                pads = xnTs[half].rearrange(
                    "p j (s n) -> p j s n", n=NP)[:, :, :, N:N + 1]
                nc.vector.tensor_copy(
                    pads,
                    zero24[:, 0:6 * SL].rearrange(
                        "p (j s) -> p j s", j=6).unsqueeze(3))

            es_ln.close()

            # ---------------- attention pairs, both halves ----------------
            # Cross-pair software pipeline: while pair p's (s,head) steps run
            # score->exp->PV, pair p+1's q/k projection GEMMs are interleaved
            # into the same tensor stream so the PE never waits on the scalar
            # engine's exp and stays at full clock.
            es_at = ExitStack()
            qk_pool = es_at.enter_context(tc.tile_pool(name="qk", bufs=2))
            e_pool = es_at.enter_context(tc.tile_pool(name="epool", bufs=3))
            rec_pool = es_at.enter_context(tc.tile_pool(name="rec", bufs=3))
            sc_pool = es_at.enter_context(
                tc.tile_pool(name="ps_sc", bufs=3, space="PSUM"))
            acc_pool = es_at.enter_context(
                tc.tile_pool(name="ps_acc", bufs=2, space="PSUM"))

            def pairs(half):
                xnT = xnTs[half]
                oall = oalls[half]
                vnat = vnats[half]
                qkTs = {}
                et1s = {}

                def new_qkT(p):
                    qT = qk_pool.tile([P, QTL], f32r, tag="qT", name="qT")
                    kT = qk_pool.tile([P, QTL], f32r, tag="kT", name="kT")
                    qkTs[p] = (qT, kT)

                def pq_group(p, g):
                    # g 0..3: k of sample g; g 4..7: q of sample g-4
                    qT, kT = qkTs[p]
                    if g < 4:
                        dst, cbase, s = kT, C + p * P, g
                    else:
                        dst, cbase, s = qT, p * P, g - 4
                    sct = sc_pool.tile([P, 1024], f32, tag="sc", name="pq")
                    for kk in range(6):
                        nc.tensor.matmul(
                            sct[:, 0:NP],
                            lhsT=wqk_sb[:, kk, cbase:cbase + P],
                            rhs=xnT[:, kk, s * NP:(s + 1) * NP],
                            start=(kk == 0), stop=(kk == 5))
                    nc.vector.tensor_copy(dst[:, s * NP:(s + 1) * NP],
                                          sct[:, 0:NP])
                    return sct

                def emit_et1_row(p, s, hh, host):
                    # 257th-key scores for (pair, sample, head) in the spare
                    # PSUM bank of this step's pq tile
                    if host is None:
                        host = sc_pool.tile([P, 1024], f32, tag="sc",
                                            name="et1h")
                    qT, kT = qkTs[p]
                    kTr = kT.rearrange("p (s n) -> p s n", n=NP)
                    rlo = hh * 64
                    nc.tensor.matmul(
                        host[0:1, 512:512 + NP],
                        lhsT=kTr[rlo:rlo + 64, s, 256:257],
                        rhs=qT[rlo:rlo + 64, s * NP:(s + 1) * NP],
                        start=True, stop=True)
                    et1 = e_pool.tile([1, NP], f32r, tag="et1", bufs=8,
                                      name="et1")
                    nc.scalar.activation(et1, host[0:1, 512:512 + NP],
                                         AF.Exp, scale=HD ** -0.5)
                    et1s[(p, hh, s)] = et1

                def emit_pv(p, ent):
                    s, hh, et = ent
                    h = 2 * p + hh
                    po = acc_pool.tile([65, NP], f32, tag="po", name="po")
                    nc.tensor.matmul(po, lhsT=vnat[(s, 0)][:, h, :],
                                     rhs=et[:, 0, :], start=True, stop=False)
                    nc.tensor.matmul(po, lhsT=vnat[(s, 1)][:, h, :],
                                     rhs=et[:, 1, :], start=False, stop=False)
                    nc.tensor.matmul(po, lhsT=vnat[(s, 2)][:, h, :],
                                     rhs=et1s[(p, hh, s)],
                                     start=False, stop=True)
                    rec = rec_pool.tile([65, NP], f32red, tag="rec",
                                        name="rec")
                    nc.vector.reciprocal(rec[64:65], po[64:65])
                    osl = oall[hh * 64:hh * 64 + 64, p, s * N:(s + 1) * N]
                    nc.vector.tensor_copy(osl, po[0:64, 0:N])
                    return (osl, rec)

                def emit_norm(ent):
                    osl, rec = ent
                    pRt = sc_pool.tile([P, 1024], f32, tag="sc", name="pRt")
                    pR = pRt[0:64, 0:NP]
                    nc.tensor.matmul(pR, lhsT=ones_t[64:65, 0:64],
                                     rhs=rec[64:65], start=True, stop=True)
                    nc.vector.tensor_mul(osl, osl, pR[:, 0:N])

                # prologue: pair 0's q/k plus its 257th-key rows
                new_qkT(0)
                for g in range(8):
                    pq_group(0, g)
                for s in range(SL):
                    for hh in range(2):
                        emit_et1_row(0, s, hh, None)

                for p in range(6):
                    if p < 5:
                        new_qkT(p + 1)
                    pv_q = None
                    norm_q = None
                    for i in range(8):
                        s, hh = i // 2, i % 2
                        sct = sc_pool.tile([P, 1024], f32, tag="sc",
                                           name="sct")
                        qT, kT = qkTs[p]
                        rlo = hh * 64
                        nc.tensor.matmul(
                            sct[:, 0:NP],
                            lhsT=kT[rlo:rlo + 64, s * NP:s * NP + 128],
                            rhs=qT[rlo:rlo + 64, s * NP:(s + 1) * NP],
                            start=True, stop=True)
                        nc.tensor.matmul(
                            sct[:, 512:512 + NP],
                            lhsT=kT[rlo:rlo + 64, s * NP + 128:s * NP + 256],
                            rhs=qT[rlo:rlo + 64, s * NP:(s + 1) * NP],
                            start=True, stop=True)
                        et = e_pool.tile([P, 2, NP], f32r, tag="et",
                                         name="et")
                        nc.scalar.activation(et[:, 0, :], sct[:, 0:NP],
                                             AF.Exp, scale=HD ** -0.5)
                        nc.scalar.activation(et[:, 1, :],
                                             sct[:, 512:512 + NP],
                                             AF.Exp, scale=HD ** -0.5)
                        host = pq_group(p + 1, i) if p < 5 else None
                        # one 257th-key row per step, two pairs in flight
                        if i < 4:
                            if p >= 1:
                                emit_et1_row(p, 2 + i // 2, i % 2, host)
                        elif p < 5:
                            emit_et1_row(p + 1, (i - 4) // 2, i % 2, host)
                        if pv_q is not None:
                            ent = emit_pv(p, pv_q)
                            if norm_q is not None:
                                emit_norm(norm_q)
                            norm_q = ent
                        pv_q = (s, hh, et)
                    ent = emit_pv(p, pv_q)
                    if norm_q is not None:
                        emit_norm(norm_q)
                    emit_norm(ent)

            pairs(0)
            pairs(1)
            es_at.close()
            es_aw.close()

            # ---------------- proj + LN2, both halves ----------------
            es_late = ExitStack()
            wm_pool = es_late.enter_context(tc.tile_pool(name="wmlp", bufs=1))
            x2_pool = es_late.enter_context(tc.tile_pool(name="x2sb", bufs=1))
            xn2_pool = es_late.enter_context(tc.tile_pool(name="xn2sb", bufs=1))

            w1_sb = wm_pool.tile([P, 6, HID], f32r, tag="w1")
            nc.gpsimd.dma_start(out=w1_sb,
                                in_=wfc1_d.rearrange("(j p) n -> p j n", p=P))
            w2_sb = wm_pool.tile([P, 24, C], f32r, tag="w2")
            nc.gpsimd.dma_start(out=w2_sb,
                                in_=wfc2_d.rearrange("(j p) n -> p j n", p=P))
            b2t_b = wm_pool.tile([P, C], f32, tag="b2t_b")
            small_scale3(b2t_b, b2_b,
                         [((0, 256), g012c), ((256, 384), g01c),
                          ((384, 768), g0c)])

            es_proj = ExitStack()
            wp1_pool = es_proj.enter_context(tc.tile_pool(name="wp1", bufs=1))
            pr_pool = es_proj.enter_context(tc.tile_pool(name="prtmp", bufs=3))
            psp_pool = es_proj.enter_context(
                tc.tile_pool(name="ps_p", bufs=4, space="PSUM"))
            ln2_pool = es_proj.enter_context(tc.tile_pool(name="ln2", bufs=3))
            pst2_pool = es_proj.enter_context(
                tc.tile_pool(name="ps_t2", bufs=2, space="PSUM"))

            wt_sb = wp1_pool.tile([P, 6, C], f32r, tag="wtilde")
            for j in range(6):
                if j < 2:
                    small_scale3(wt_sb[:, j], wproj_sb[:, j],
                                 [((0, 256), g012c), ((256, 384), g01c),
                                  ((384, 768), g0c)])
                elif j == 2:
                    small_scale3(wt_sb[:, j], wproj_sb[:, j],
                                 [((0, 384), g01c), ((384, 768), g0c)])
                else:
                    small_scale3(wt_sb[:, j], wproj_sb[:, j],
                                 [((0, 768), g0c)])
            bt_b = wp1_pool.tile([P, C], f32, tag="btilde")
            small_scale3(bt_b, bp_b,
                         [((0, 256), g012c), ((256, 384), g01c),
                          ((384, 768), g0c)])
            bt_row = wp1_pool.tile([1, C], f32r, tag="bt_row")
            nc.vector.tensor_copy(bt_row, bt_b[0:1])

            x2alls = {}
            for half in (0, 1):
                x2alls[half] = x2_pool.tile([P, len(TCH), C], f32r,
                                            tag=f"x2all{half}",
                                            name=f"x2all{half}")
            xn2Ts = {}

            def proj_mm(half):
                oall = oalls[half]
                wp_eff = wproj_sb if half == 0 else wt_sb
                bp_eff = bp_row if half == 0 else bt_row
                x2all = x2alls[half]
                for i, (of, sz) in enumerate(TCH):
                    x_t = pr_pool.tile([P, C], f32, tag="resx")
                    nc.sync.dma_start(
                        out=x_t[0:sz, 0:384],
                        in_=x_flat[half * T + of:half * T + of + sz, 0:384])
                    nc.scalar.dma_start(
                        out=x_t[0:sz, 384:768],
                        in_=x_flat[half * T + of:half * T + of + sz, 384:768])
                    for ch, (ca, cw) in enumerate(CCH):
                        pp = psp_pool.tile([P, 384], f32, tag="psp")
                        for dk in range(6):
                            nc.tensor.matmul(
                                pp[0:sz],
                                lhsT=oall[:, dk, of:of + sz],
                                rhs=wp_eff[:, dk, ca:ca + cw],
                                start=(dk == 0), stop=False)
                        nc.tensor.matmul(
                            pp[0:sz], lhsT=ones_row[:, 0:sz],
                            rhs=bp_eff[:, ca:ca + cw],
                            start=False, stop=True)
                        nc.vector.tensor_add(x2all[0:sz, i, ca:ca + cw],
                                             pp[0:sz], x_t[0:sz, ca:ca + cw])

            def ln2_phase(half):
                gc = lncols["n2l_g" if half == 0 else "n2s_g"]
                bc = lncols["n2l_b" if half == 0 else "n2s_b"]
                xn2T = xn2_pool.tile([P, 6, T], f32r, tag=f"xn2T{half}",
                                     name=f"xn2T{half}")
                xn2Ts[half] = xn2T
                for i, (of, sz) in enumerate(TCH):
                    x2_t = x2alls[half][0:sz, i]
                    xg2 = x2_t.rearrange("p (g d) -> p g d", g=3)
                    stats = ln2_pool.tile([P, 3, 6], f32, tag="ln2_stats")
                    for gi in range(3):
                        nc.vector.bn_stats(out=stats[0:sz, gi], in_=xg2[:, gi])
                    mv = ln2_pool.tile([P, 2], f32, tag="ln2_mv")
                    nc.vector.bn_aggr(out=mv[0:sz], in_=stats[0:sz])
                    rstd = ln2_pool.tile([P, 1], f32, tag="ln2_rstd")
                    nc.scalar.activation(rstd[0:sz], mv[0:sz, 1:2], AF.Sqrt,
                                         bias=eps_t[0:sz], scale=1.0)
                    nc.vector.reciprocal(rstd[0:sz], rstd[0:sz])
                    pre = ln2_pool.tile([P, C], f32r, tag="ln2_pre")
                    nc.vector.tensor_scalar(pre[0:sz], x2_t,
                                            scalar1=mv[0:sz, 0:1],
                                            scalar2=rstd[0:sz],
                                            op0=OP.subtract, op1=OP.mult)
                    for j in range(6):
                        pst = pst2_pool.tile([P, P], f32r, tag="pst2")
                        nc.tensor.transpose(pst[:, 0:sz],
                                            pre[0:sz, j * P:(j + 1) * P],
                                            ident_b[0:sz, 0:sz])
                        nc.scalar.activation(xn2T[:, j, of:of + sz],
                                             pst[:, 0:sz], AF.Identity,
                                             bias=bc[:, j:j + 1],
                                             scale=gc[:, j:j + 1])

            proj_mm(0)
            ln2_phase(0)
            proj_mm(1)
            ln2_phase(1)
            es_proj.close()

            # ---------------- MLP, both halves ----------------
            es_mlp = ExitStack()
            hs_pool = es_mlp.enter_context(tc.tile_pool(name="hsm", bufs=2))
            mo_pool = es_mlp.enter_context(tc.tile_pool(name="mout", bufs=3))
            psf_pool = es_mlp.enter_context(
                tc.tile_pool(name="ps_f", bufs=2, space="PSUM"))
            psout_pool = es_mlp.enter_context(
                tc.tile_pool(name="ps_out", bufs=1, space="PSUM"))

            for half in (0, 1):
                b2_eff = b2_b if half == 0 else b2t_b
                xn2T = xn2Ts[half]
                for (gof, chunks) in GRPS:
                    gsz = sum(s for _, s in chunks)
                    xg = xn2T[:, :, gof:gof + gsz]
                    pso = [[psout_pool.tile([P, 384], f32, tag=f"pso_{i}_{ch}",
                                            name=f"pso_{i}_{ch}")
                            for ch in range(2)] for i in range(len(chunks))]

                    def emit_fc2(hstuff, m):
                        if half == 0:
                            hrow = hstuff
                            for i, (tco, tcs) in enumerate(chunks):
                                for ch, (ca, cw) in enumerate(CCH):
                                    nc.tensor.matmul(
                                        pso[i][ch][0:tcs],
                                        lhsT=hrow[:, tco:tco + tcs],
                                        rhs=w2_sb[:, m, ca:ca + cw],
                                        start=(m == 0), stop=(m == 23))
                        else:
                            HAt, HBt, h0t = hstuff
                            for i, (tco, tcs) in enumerate(chunks):
                                tsl = slice(tco, tco + tcs)
                                # HBt shares a PSUM bank with HAt's region;
                                # only the first (HAt) matmul may start it.
                                for src, ch, ca, cw, st in (
                                        (HAt, 0, 0, 256, m == 0),
                                        (HBt, 0, 256, 128, False),
                                        (h0t, 1, 384, 384, m == 0)):
                                    nc.tensor.matmul(
                                        pso[i][ch][0:tcs, ca - ch * 384:
                                                   ca - ch * 384 + cw],
                                        lhsT=src[:, tsl],
                                        rhs=w2_sb[:, m, ca:ca + cw],
                                        start=st, stop=(m == 23),
                                        skip_group_check=True)

                    hprev = None
                    for m in range(24):
                        msl = slice(m * P, (m + 1) * P)
                        pf = psf_pool.tile([P, 384], f32, tag="psf")
                        if half == 0:
                            for kk in range(6):
                                nc.tensor.matmul(
                                    pf[:, 0:gsz],
                                    lhsT=w1_sb[:, kk, msl],
                                    rhs=xg[:, kk],
                                    start=(kk == 0), stop=(kk == 5))
                            hrow = hs_pool.tile([P, 384], f32r, tag="hrow")
                            nc.scalar.activation(hrow[:, 0:gsz], pf[:, 0:gsz],
                                                 AF.Gelu,
                                                 bias=b1_col[:, m:m + 1],
                                                 scale=1.0)
                            cur = hrow
                        else:
                            c2 = hs_pool.tile([P, 384], f32, tag="c2")
                            c1 = hs_pool.tile([P, 384], f32, tag="c1")
                            for kk in range(6):
                                nc.tensor.matmul(
                                    pf[:, 0:gsz],
                                    lhsT=w1_sb[:, kk, msl],
                                    rhs=xg[:, kk],
                                    start=(kk == 0),
                                    stop=(kk in (1, 2, 5)),
                                    skip_group_check=(kk >= 2))
                                if kk == 1:
                                    nc.vector.tensor_copy(c2[:, 0:gsz],
                                                          pf[:, 0:gsz])
                                elif kk == 2:
                                    nc.vector.tensor_copy(c1[:, 0:gsz],
                                                          pf[:, 0:gsz])
                            h2t = hs_pool.tile([P, 384], f32r, tag="h2t")
                            nc.scalar.activation(h2t[:, 0:gsz], c2[:, 0:gsz],
                                                 AF.Gelu,
                                                 bias=b1_col[:, m:m + 1],
                                                 scale=1.0)
                            h1t = hs_pool.tile([P, 384], f32r, tag="h1t")
                            nc.scalar.activation(h1t[:, 0:gsz], c1[:, 0:gsz],
                                                 AF.Gelu,
                                                 bias=b1_col[:, m:m + 1],
                                                 scale=1.0)
                            h0t = hs_pool.tile([P, 384], f32r, tag="h0t")
                            nc.scalar.activation(h0t[:, 0:gsz], pf[:, 0:gsz],
                                                 AF.Gelu,
                                                 bias=b1_col[:, m:m + 1],
                                                 scale=1.0)
                            nc.vector.tensor_scalar_mul(h0t[:, 0:gsz],
                                                        h0t[:, 0:gsz], g0c)
                            HBt = hs_pool.tile([P, 384], f32r, tag="HBt")
                            nc.vector.scalar_tensor_tensor(
                                HBt[:, 0:gsz], in0=h1t[:, 0:gsz], scalar=g1c,
                                in1=h0t[:, 0:gsz], op0=OP.mult, op1=OP.add)
                            HAt = hs_pool.tile([P, 384], f32r, tag="HAt")
                            nc.vector.scalar_tensor_tensor(
                                HAt[:, 0:gsz], in0=h2t[:, 0:gsz], scalar=g2c,
                                in1=HBt[:, 0:gsz], op0=OP.mult, op1=OP.add)
                            cur = (HAt, HBt, h0t)
                        if hprev is not None:
                            emit_fc2(hprev[0], hprev[1])
                        hprev = (cur, m)
                    emit_fc2(hprev[0], 23)

                    for i, (tco, tcs) in enumerate(chunks):
                        of = gof + tco
                        ig = of // P
                        ev = mo_pool.tile([P, C], f32, tag="mo_ev")
                        for ch, (ca, cw) in enumerate(CCH):
                            nc.vector.tensor_add(ev[0:tcs, ca:ca + cw],
                                                 pso[i][ch][0:tcs],
                                                 b2_eff[0:tcs, ca:ca + cw])
                        ot = mo_pool.tile([P, C], f32, tag="mo_out")
                        nc.vector.tensor_add(ot[0:tcs], ev[0:tcs],
                                             x2alls[half][0:tcs, ig])
                        eng = (nc.sync, nc.scalar, nc.gpsimd)[i % 3]
                        eng.dma_start(
                            out=out_flat[half * T + of:half * T + of + tcs],
                            in_=ot[0:tcs])

            es_mlp.close()
            es_late.close()
            es_w0.close()
            es_oall.close()

    nc.compile()
    return nc


def _get_nc():
    if "nc" not in _CACHE:
        _CACHE["nc"] = _build()
    return _CACHE["nc"]


def kernel(**inputs):
    from concourse import bass_utils

    nc = _get_nc()
    arrs = {k: np.ascontiguousarray(np.asarray(v, dtype=np.float32))
            for k, v in inputs.items()}
    x = arrs.pop("x")
    B = x.shape[0]
    B2 = B // 2
    per = B2 // NCORES
    in_maps = []
    for c in range(NCORES):
        shard = np.concatenate([x[c * per:(c + 1) * per],
                                x[B2 + c * per:B2 + (c + 1) * per]], axis=0)
        m = {"x": np.ascontiguousarray(shard)}
        m.update(arrs)
        in_maps.append(m)
    res = bass_utils.run_bass_kernel_spmd(nc, in_maps,
                                          core_ids=list(range(NCORES)))
    out = np.empty_like(x.reshape(B, N, C))
    for c in range(NCORES):
        o = res.results[c]["out"]
        out[c * per:(c + 1) * per] = o[:per]
        out[B2 + c * per:B2 + (c + 1) * per] = o[per:]
    return out
